# revision 24
# baseline (speedup 1.0000x reference)
"""Trainium2 Bass kernel for nn_BaselineMNISTClassifier (vq_codebook).

reference:
    x = samples - 0.5                        # [B, F]
    hv = einsum('bf,df->bd', x, bhv)         # [B, D]
    e = (hv > 0)                             # binary
    ham[b, c] = sum_d |e - centroids[c, d]|  # [B, C]
    return -ham

Only the SIGN of hv survives the binarize, so the encode matmul runs in
fp8 e4m3 (measured 0.91% bit-flip rate vs the f32 reference -> final
rel err ~8e-3, well under the 2e-2 gate). fp8 enables the PE's
DoubleRow perf mode: 256 contraction rows per matmul at 2 rows/cycle,
i.e. 4x the f32r rate the previous version used (256 cycles for a
256x128x512 matmul vs 512 cycles for 128x128x512).

Identity for the Hamming stage: with e' = (hv > 0) - 0.5 in {+-1/2} and
cmod = 1 - 2c in {-1, 0, +1}:  |e - c| = e' * cmod + 1/2, so
    ham[b, c] = sum_d e'[b, d] * cmod[c, d] + D/2
a second tiny matmul over the same d-tiles, also fp8 DoubleRow (exact:
all values are +-0.5/+-1/0). The binarize alternates DVE (is_gt-sub ->
e' = +-0.5) and ACT (Sign -> +-1) per d-tile; the matching 1.0 / 0.5
cmod scale is baked into the host-prepared centroid weights per d-tile
so both conventions contribute identically.

Sharding: D axis (10000) split across 8 cores, 1250 (padded 1280) per
core; every core sees the full batch, partial hammings sum on host.
F = 784 is zero-padded to 1024 = 4 chunks x (2 ktiles x 128 rows) so
every encode matmul is a uniform full-width DoubleRow op.

All quantization/transposition happens on host: x8 = fp8(64*(x-0.5)),
w8 = fp8(64*w) (the 64x scaling keeps values away from fp8 subnormals;
sign(hv) is scale-invariant), cmod in fp8 exactly.

Perf structure (per core):
  - warmup matmuls release the PE HAM clock gate while inputs stream
  - 2 b-groups of 4 blocks x 512; per (d-tile, block) 4 DoubleRow
    matmuls (q-chunks) accumulate in one PSUM bank; 7-bank rotation
  - consecutive matmuls over the 4 blocks share stationary weights,
    hiding LDWEIGHTS
  - hamming matmuls for d-pair P issue one d-tile late so the PE never
    waits on the binarize; all 4 accumulators of a b-group live in ONE
    PSUM bank at partition offsets 0/32/64/96 (tile_position)
  - last d-tile binarizes in halves; final hamming + epilogue drain
    per-block, outputs DMA out as their accumulation closes
"""

import sys

sys.path.insert(0, "/opt/trn_rl_repo")

import ml_dtypes
import numpy as np

import concourse.bacc as bacc
import concourse.bass as bass
import concourse.mybir as mybir
import concourse.tile as tile
from concourse.bass_utils import run_bass_kernel_spmd

B = 4096
F = 784
FP = 1024                    # F zero-padded: 4 chunks x (2 ktiles x 128)
NQ = 4                       # k-chunks of 256 (DoubleRow contraction)
D = 10000
C = 10
NCORES = 8
DREAL = D // NCORES          # 1250 real dims per core
DP = 1280                    # padded to 10 d-tiles of 128
ND = DP // 128               # 10
NPAIR = ND // 2              # 5 hamming d-pairs
NB = B // 512                # 8 b-blocks of 512
HC = 16                      # hamming stationary cols padded 10 -> 16
NWARM = 10                   # PE warmup matmuls
NG = 4                       # b-subgroups of 2 blocks of 512

F32 = mybir.dt.float32
FP8 = mybir.dt.float8e4
OP = mybir.AluOpType
AF = mybir.ActivationFunctionType
PM = mybir.MatmulPerfMode

NP_FP8 = ml_dtypes.float8_e4m3
XSCALE = 64.0

# binarize engine per b-block: even blocks on DVE (is_gt-sub, e' = +-0.5,
# cmod +-1), odd blocks on ACT (Sign, e2 = +-1, cmod +-0.5). The host
# provides both cmod scalings per d-tile so each hamming matmul picks the
# variant matching its block's binarize convention.

_NC_CACHE = {}


def _build_nc():
    if "nc" in _NC_CACHE:
        return _NC_CACHE["nc"]
    nc = bacc.Bacc("TRN2", debug=False, target_bir_lowering=False)
    # x8 rows [g*128 + r]: one SBUF partition's full 8KB for b-subgroup
    # g (contiguous -> one DMA descriptor per partition)
    x8 = nc.dram_tensor("x8", [NG * 128, NQ * 2 * 1024], FP8,
                        kind="ExternalInput")
    # stationary operands come pre-interleaved for DoubleRowSwInterleave:
    # per partition, position 2*(M-1-m)+t holds the ktile-t weight of
    # output column m (the HW dual-fp8 LDWEIGHTS layout, probe-verified)
    w8 = nc.dram_tensor("w8", [128, ND * NQ * 256], FP8,
                        kind="ExternalInput")
    cm8 = nc.dram_tensor("cm8", [128, ND * 2 * HC], FP8,
                         kind="ExternalInput")
    out = nc.dram_tensor("out", [C, B], F32, kind="ExternalOutput")

    with tile.TileContext(nc) as tc:
        with (
            tc.tile_pool(name="dum", bufs=2) as dumpool,
            tc.tile_pool(name="xp", bufs=2) as xpool,
            tc.tile_pool(name="wp", bufs=1) as wpool,
            tc.tile_pool(name="cp", bufs=1) as cpool,
            tc.tile_pool(name="ep", bufs=12) as epool,
            tc.tile_pool(name="op", bufs=4) as opool,
            tc.tile_pool(name="pse", bufs=6, space="PSUM") as psepool,
            tc.tile_pool(name="ps2", bufs=2, space="PSUM") as ps2pool,
        ):
            # --- input loads: the first b-subgroup's x chunk (1MB) and
            # the first w chunk are the critical path to the first encode
            # matmul; everything else is deferred behind compute progress
            # (write/read dependencies keep the scheduler from hoisting
            # the triggers into the critical stream)
            xgs = [xpool.tile([128, NQ, 2, 1024], FP8, name=f"xg{g}",
                              tag="xt") for g in range(NG)]

            def load_xg(g, eng):
                eng.dma_start(
                    xgs[g][:], x8[g * 128:(g + 1) * 128, :].rearrange(
                        "p (q t c) -> p q t c", q=NQ, t=2))

            load_xg(0, nc.sync)
            wt = wpool.tile([128, ND, NQ, 256], FP8)

            def load_w(lo, hi, eng):
                eng.dma_start(
                    wt[:, lo:hi, :, :],
                    w8[:, lo * NQ * 256:hi * NQ * 256].rearrange(
                        "p (a q m) -> p a q m", a=hi - lo, q=NQ))

            load_w(0, 4, nc.gpsimd)
            ct = cpool.tile([128, ND, 2, HC], FP8)
            nc.gpsimd.dma_start(
                ct[:], cm8.ap().rearrange("p (a v m) -> p a v m",
                                          a=ND, v=2))
            # w d4-9 waits on the xg0 load (1-elem copy creates the dep),
            # so it streams only after the critical chunk has landed
            nc.gpsimd.tensor_scalar_mul(wt[0:1, 4, 0, 0:1],
                                        xgs[0][0:1, 0, 0, 0:1], 0.0)
            load_w(4, ND, nc.gpsimd)

            # --- PE warmup: ramp the PE clock while inputs stream; the
            # dummies memset on DVE so the GpSimd queue stays free for
            # DMA triggers
            wdum = dumpool.tile([128, 256], FP8)
            nc.vector.memset(wdum[:], 1.0)
            xdum = dumpool.tile([128, 2, 512], FP8)
            nc.vector.memset(xdum[:], 1.0)
            psdum = psepool.tile([128, 512], F32, name="psdum", tag="pse")
            for i in range(NWARM):
                nc.tensor.matmul(psdum[:], wdum[:], xdum[:],
                                 start=(i == 0), stop=(i == NWARM - 1),
                                 perf_mode=PM.DoubleRowSwInterleave)

            # --- main compute: four b-subgroups of 2 blocks of 512.
            for g in range(NG):
                ps2 = ps2pool.tile([128, 512], F32, name=f"ps2_{g % 2}",
                                   tag="ps2")
                psum2 = [ps2[32 * jj:32 * jj + HC, :] for jj in range(2)]
                psum2o = [ps2[32 * jj:32 * jj + C, :] for jj in range(2)]
                pending = []
                for di in range(ND):
                    pses = [
                        psepool.tile([128, 512], F32,
                                     name=f"pse_{di % 3}_{jj}", tag="pse")
                        for jj in range(2)
                    ]
                    for q in range(NQ):
                        wq = wt[:, di, q, :]
                        for jj in range(2):
                            nc.tensor.matmul(
                                pses[jj][:], wq,
                                xgs[g][:, q, :, jj * 512:(jj + 1) * 512],
                                start=(q == 0), stop=(q == NQ - 1),
                                perf_mode=PM.DoubleRowSwInterleave)
                    # hamming for the previous d-tile: issued here so the
                    # PE reaches it well after its binarize completes
                    for pdi, pets in pending:
                        for jj in range(2):
                            nc.tensor.matmul(
                                psum2[jj], ct[:, pdi, jj, :], pets[jj][:],
                                start=(pdi == 0), stop=False,
                                tile_position=(0, 32 * jj))
                    pending = []
                    # binarize this d-tile: even block on DVE (e'=+-0.5),
                    # odd block on ACT Sign (+-1); the last d-tile goes in
                    # halves so the final hamming overlaps
                    ets = [epool.tile([128, 512], FP8,
                                      name=f"et_{di % 3}_{jj}", tag="et")
                           for jj in range(2)]
                    for jj in range(2):
                        sls = ([slice(0, 256), slice(256, 512)]
                               if di == ND - 1 else [slice(0, 512)])
                        for sl in sls:
                            if jj == 0:
                                nc.vector.tensor_scalar(
                                    ets[jj][:, sl], pses[jj][:, sl],
                                    0.0, 0.5,
                                    op0=OP.is_gt, op1=OP.subtract)
                            else:
                                nc.scalar.activation(
                                    ets[jj][:, sl], pses[jj][:, sl],
                                    AF.Sign)
                    if g < NG - 1 and di == 1:
                        # release the next subgroup's x chunk only now: a
                        # 1-elem write into it (reading this d-tile's
                        # psum) gives its DMA a write dependency, keeping
                        # the scheduler from hoisting the trigger into the
                        # critical early stream
                        nc.vector.tensor_scalar_mul(
                            xgs[g + 1][0:1, 0, 0, 0:1],
                            pses[0][0:1, 0:1], 0.0)
                        load_xg(g + 1, nc.scalar)
                    pending.append((di, ets))
                # final d-tile: hamming in halves, epilogue + output DMA
                # per block as each accumulation closes
                pdi, pets = pending[0]
                for jj in range(2):
                    for h in range(2):
                        sl = slice(h * 256, (h + 1) * 256)
                        nc.tensor.matmul(
                            psum2[jj][:, sl], ct[:, pdi, jj, :],
                            pets[jj][:, sl],
                            start=False, stop=True,
                            tile_position=(0, 32 * jj))
                    # out = -(psum2 + DREAL/2); alternate engines so the
                    # epilogues drain in parallel
                    ot = opool.tile([C, 512], F32, name=f"ot_{jj}",
                                    tag="ot")
                    if jj == 0:
                        nc.vector.tensor_scalar(ot[:], psum2o[jj],
                                                float(DREAL) / 2.0, -1.0,
                                                op0=OP.add, op1=OP.mult)
                    else:
                        nc.scalar.activation(ot[:], psum2o[jj], AF.Copy,
                                             bias=-float(DREAL) / 2.0,
                                             scale=-1.0)
                    nc.gpsimd.dma_start(
                        out[:, (g * 2 + jj) * 512:(g * 2 + jj + 1) * 512],
                        ot[:])
    nc.compile()
    _NC_CACHE["nc"] = nc
    return nc


def _prep_in_maps(samples, bhv_matrix, centroids):
    samples = np.ascontiguousarray(samples, dtype=np.float32)
    bhv_matrix = np.ascontiguousarray(bhv_matrix, dtype=np.float32)
    centroids = np.ascontiguousarray(centroids, dtype=np.float32)

    # x8 [NG*128, 8KB]: [g*128 + r, (q*2+t)*1024 + c] =
    # fp8(64*(x - 0.5))[f = q*256+t*128+r, b = g*1024+c], 0-padded
    xz = np.zeros((FP, B), dtype=np.float32)
    xz[:F, :] = (samples.T - 0.5) * XSCALE
    xv = xz.reshape(NQ, 2, 128, NG, 1024).transpose(3, 2, 0, 1, 4)
    x_all = np.ascontiguousarray(xv).reshape(NG * 128, NQ * 2 * 1024)
    x_all = x_all.astype(NP_FP8)

    in_maps = []
    for k in range(NCORES):
        lo, hi = k * DREAL, (k + 1) * DREAL
        wz = np.zeros((FP, DP), dtype=np.float32)
        wz[:F, :DREAL] = bhv_matrix[lo:hi, :].T * XSCALE
        # SwInterleave stationary layout: [r, q, di, m, t] with m reversed,
        # flattened so position 2*(127-m)+t holds (f=q*256+t*128+r, d=di*128+m)
        wv = wz.reshape(NQ, 2, 128, ND, 128).transpose(2, 3, 0, 4, 1)
        wv = wv[:, :, :, ::-1, :]
        w8 = np.ascontiguousarray(wv).reshape(128, ND * NQ * 256)
        w8 = w8.astype(NP_FP8)
        cz = np.zeros((DP, 2, HC), dtype=np.float32)
        cz[:DREAL, 0, :C] = 1.0 - 2.0 * centroids[:, lo:hi].T
        cz[:, 1, :] = 0.5 * cz[:, 0, :]
        cv = cz.reshape(ND, 128, 2, HC).transpose(1, 0, 2, 3)
        cm8 = np.ascontiguousarray(cv).reshape(128, ND * 2 * HC)
        cm8 = cm8.astype(NP_FP8)
        in_maps.append({"x8": x_all, "w8": w8, "cm8": cm8})
    return in_maps


def _run(samples, bhv_matrix, centroids, **spmd_kwargs):
    nc = _build_nc()
    in_maps = _prep_in_maps(samples, bhv_matrix, centroids)
    res = run_bass_kernel_spmd(nc, in_maps, core_ids=list(range(NCORES)),
                               **spmd_kwargs)
    acc = np.zeros((C, B), dtype=np.float32)
    for r in res.results:
        acc += r["out"]
    return np.ascontiguousarray(acc.T), res


def kernel(samples, bhv_matrix, centroids):
    out, _ = _run(samples, bhv_matrix, centroids)
    return out


# revision 26
# speedup vs baseline: 1.0217x; 1.0217x over previous
"""Trainium2 Bass kernel for nn_BaselineMNISTClassifier (vq_codebook).

reference:
    x = samples - 0.5                        # [B, F]
    hv = einsum('bf,df->bd', x, bhv)         # [B, D]
    e = (hv > 0)                             # binary
    ham[b, c] = sum_d |e - centroids[c, d]|  # [B, C]
    return -ham

Only the SIGN of hv survives the binarize, so the encode matmul runs in
fp8 e4m3 (measured 0.91% bit-flip rate vs the f32 reference -> final
rel err ~8e-3, well under the 2e-2 gate). fp8 enables the PE's
DoubleRow perf mode: 256 contraction rows per matmul at 2 rows/cycle,
i.e. 4x the f32r rate the previous version used (256 cycles for a
256x128x512 matmul vs 512 cycles for 128x128x512).

Identity for the Hamming stage: with e' = (hv > 0) - 0.5 in {+-1/2} and
cmod = 1 - 2c in {-1, 0, +1}:  |e - c| = e' * cmod + 1/2, so
    ham[b, c] = sum_d e'[b, d] * cmod[c, d] + D/2
a second tiny matmul over the same d-tiles, also fp8 DoubleRow (exact:
all values are +-0.5/+-1/0). The binarize alternates DVE (is_gt-sub ->
e' = +-0.5) and ACT (Sign -> +-1) per d-tile; the matching 1.0 / 0.5
cmod scale is baked into the host-prepared centroid weights per d-tile
so both conventions contribute identically.

Sharding: D axis (10000) split across 8 cores, 1250 (padded 1280) per
core; every core sees the full batch, partial hammings sum on host.
F = 784 is zero-padded to 1024 = 4 chunks x (2 ktiles x 128 rows) so
every encode matmul is a uniform full-width DoubleRow op.

All quantization/transposition happens on host: x8 = fp8(64*(x-0.5)),
w8 = fp8(64*w) (the 64x scaling keeps values away from fp8 subnormals;
sign(hv) is scale-invariant), cmod in fp8 exactly.

Perf structure (per core):
  - warmup matmuls release the PE HAM clock gate while inputs stream
  - 2 b-groups of 4 blocks x 512; per (d-tile, block) 4 DoubleRow
    matmuls (q-chunks) accumulate in one PSUM bank; 7-bank rotation
  - consecutive matmuls over the 4 blocks share stationary weights,
    hiding LDWEIGHTS
  - hamming matmuls for d-pair P issue one d-tile late so the PE never
    waits on the binarize; all 4 accumulators of a b-group live in ONE
    PSUM bank at partition offsets 0/32/64/96 (tile_position)
  - last d-tile binarizes in halves; final hamming + epilogue drain
    per-block, outputs DMA out as their accumulation closes
"""

import sys

sys.path.insert(0, "/opt/trn_rl_repo")

import ml_dtypes
import numpy as np

import concourse.bacc as bacc
import concourse.bass as bass
import concourse.mybir as mybir
import concourse.tile as tile
from concourse.bass_utils import run_bass_kernel_spmd

B = 4096
F = 784
FP = 1024                    # F zero-padded: 4 chunks x (2 ktiles x 128)
NQ = 4                       # k-chunks of 256 (DoubleRow contraction)
D = 10000
C = 10
NCORES = 8
DREAL = D // NCORES          # 1250 real dims per core
DP = 1280                    # padded to 10 d-tiles of 128
ND = DP // 128               # 10
NPAIR = ND // 2              # 5 hamming d-pairs
NB = B // 512                # 8 b-blocks of 512
HC = 16                      # hamming stationary cols padded 10 -> 16
NWARM = 10                   # PE warmup matmuls
GROUPS = (2, 3, 3)           # b-subgroup sizes in blocks of 512
GOFF = (0, 2, 5)             # first block of each subgroup

F32 = mybir.dt.float32
FP8 = mybir.dt.float8e4
OP = mybir.AluOpType
AF = mybir.ActivationFunctionType
PM = mybir.MatmulPerfMode

NP_FP8 = ml_dtypes.float8_e4m3
XSCALE = 64.0

# binarize engine per b-block: even blocks on DVE (is_gt-sub, e' = +-0.5,
# cmod +-1), odd blocks on ACT (Sign, e2 = +-1, cmod +-0.5). The host
# provides both cmod scalings per d-tile so each hamming matmul picks the
# variant matching its block's binarize convention.

_NC_CACHE = {}


def _build_nc():
    if "nc" in _NC_CACHE:
        return _NC_CACHE["nc"]
    nc = bacc.Bacc("TRN2", debug=False, target_bir_lowering=False)
    # x8: per-subgroup blobs, each [128, NQ*2*nblk*512] partition-major
    # (contiguous -> one DMA descriptor per partition per blob)
    x8 = nc.dram_tensor("x8", [128, NQ * 2 * B], FP8,
                        kind="ExternalInput")
    # stationary operands come pre-interleaved for DoubleRowSwInterleave:
    # per partition, position 2*(M-1-m)+t holds the ktile-t weight of
    # output column m (the HW dual-fp8 LDWEIGHTS layout, probe-verified)
    w8 = nc.dram_tensor("w8", [128, ND * NQ * 256], FP8,
                        kind="ExternalInput")
    cm8 = nc.dram_tensor("cm8", [128, ND * 2 * HC], FP8,
                         kind="ExternalInput")
    out = nc.dram_tensor("out", [C, B], F32, kind="ExternalOutput")

    with tile.TileContext(nc) as tc:
        with (
            tc.tile_pool(name="dum", bufs=2) as dumpool,
            tc.tile_pool(name="xp", bufs=1) as xpool,
            tc.tile_pool(name="wp", bufs=1) as wpool,
            tc.tile_pool(name="cp", bufs=1) as cpool,
            tc.tile_pool(name="ep", bufs=12) as epool,
            tc.tile_pool(name="op", bufs=4) as opool,
            tc.tile_pool(name="pse", bufs=7, space="PSUM") as psepool,
            tc.tile_pool(name="ps2", bufs=1, space="PSUM") as ps2pool,
        ):
            # --- input loads: the first b-subgroup's x chunk (1MB) and
            # the first w chunk are the critical path to the first encode
            # matmul; everything else is deferred behind compute progress
            # (write/read dependencies keep the scheduler from hoisting
            # the triggers into the critical stream)
            xgs = [xpool.tile([128, NQ, 2, nb * 512], FP8, name=f"xg{g}",
                              tag=f"xt{g}")
                   for g, nb in enumerate(GROUPS)]

            def load_xg(g, eng):
                off = GOFF[g] * 512 * NQ * 2
                sz = GROUPS[g] * 512 * NQ * 2
                eng.dma_start(
                    xgs[g][:], x8[:, off:off + sz].rearrange(
                        "p (q t c) -> p q t c", q=NQ, t=2))

            load_xg(0, nc.sync)
            wt = wpool.tile([128, ND, NQ, 256], FP8)

            def load_w(lo, hi, eng):
                eng.dma_start(
                    wt[:, lo:hi, :, :],
                    w8[:, lo * NQ * 256:hi * NQ * 256].rearrange(
                        "p (a q m) -> p a q m", a=hi - lo, q=NQ))

            load_w(0, 4, nc.gpsimd)
            ct = cpool.tile([128, ND, 2, HC], FP8)
            nc.gpsimd.dma_start(
                ct[:], cm8.ap().rearrange("p (a v m) -> p a v m",
                                          a=ND, v=2))
            # w d4-9 waits on the xg0 load (1-elem copy creates the dep),
            # so it streams only after the critical chunk has landed
            nc.gpsimd.tensor_scalar_mul(wt[0:1, 4, 0, 0:1],
                                        xgs[0][0:1, 0, 0, 0:1], 0.0)
            load_w(4, ND, nc.gpsimd)

            # --- PE warmup: ramp the PE clock while inputs stream; the
            # dummies memset on DVE so the GpSimd queue stays free for
            # DMA triggers
            wdum = dumpool.tile([128, 256], FP8)
            nc.vector.memset(wdum[:], 1.0)
            xdum = dumpool.tile([128, 2, 512], FP8)
            nc.vector.memset(xdum[:], 1.0)
            psdum = psepool.tile([128, 512], F32, name="psdum", tag="pse")
            for i in range(NWARM):
                nc.tensor.matmul(psdum[:], wdum[:], xdum[:],
                                 start=(i == 0), stop=(i == NWARM - 1),
                                 perf_mode=PM.DoubleRowSwInterleave)

            # --- main compute: b-subgroups of GROUPS blocks of 512.
            for g, nblk in enumerate(GROUPS):
                ps2 = ps2pool.tile([128, 512], F32, name=f"ps2_{g % 2}",
                                   tag="ps2")
                psum2 = [ps2[32 * jj:32 * jj + HC, :] for jj in range(nblk)]
                psum2o = [ps2[32 * jj:32 * jj + C, :] for jj in range(nblk)]
                pending = []
                for di in range(ND):
                    pses = [
                        psepool.tile([128, 512], F32,
                                     name=f"pse_{di % 2}_{jj}", tag="pse")
                        for jj in range(nblk)
                    ]
                    for q in range(NQ):
                        wq = wt[:, di, q, :]
                        for jj in range(nblk):
                            nc.tensor.matmul(
                                pses[jj][:], wq,
                                xgs[g][:, q, :, jj * 512:(jj + 1) * 512],
                                start=(q == 0), stop=(q == NQ - 1),
                                perf_mode=PM.DoubleRowSwInterleave)
                    # hamming for the previous d-tile: issued here so the
                    # PE reaches it well after its binarize completes
                    for pdi, pets in pending:
                        for jj in range(nblk):
                            nc.tensor.matmul(
                                psum2[jj], ct[:, pdi, jj % 2, :],
                                pets[jj][:],
                                start=(pdi == 0), stop=False,
                                tile_position=(0, 32 * jj))
                    pending = []
                    # binarize this d-tile: even blocks on DVE (e'=+-0.5),
                    # odd blocks on ACT Sign (+-1); the last d-tile goes in
                    # halves so the final hamming overlaps
                    ets = [epool.tile([128, 512], FP8,
                                      name=f"et_{di % 2}_{jj}", tag="et")
                           for jj in range(nblk)]
                    for jj in range(nblk):
                        sls = ([slice(0, 256), slice(256, 512)]
                               if di == ND - 1 else [slice(0, 512)])
                        for sl in sls:
                            if jj % 2 == 0:
                                nc.vector.tensor_scalar(
                                    ets[jj][:, sl], pses[jj][:, sl],
                                    0.0, 0.5,
                                    op0=OP.is_gt, op1=OP.subtract)
                            else:
                                nc.scalar.activation(
                                    ets[jj][:, sl], pses[jj][:, sl],
                                    AF.Sign)
                    if g < len(GROUPS) - 1 and di == 1:
                        # release the next subgroup's x chunk only now: a
                        # 1-elem write into it (reading this d-tile's
                        # psum) gives its DMA a write dependency, keeping
                        # the scheduler from hoisting the trigger into the
                        # critical early stream
                        nc.vector.tensor_scalar_mul(
                            xgs[g + 1][0:1, 0, 0, 0:1],
                            pses[0][0:1, 0:1], 0.0)
                        load_xg(g + 1, nc.scalar)
                    pending.append((di, ets))
                # final d-tile: hamming in halves, epilogue + output DMA
                # per block as each accumulation closes
                pdi, pets = pending[0]
                for jj in range(nblk):
                    for h in range(2):
                        sl = slice(h * 256, (h + 1) * 256)
                        nc.tensor.matmul(
                            psum2[jj][:, sl], ct[:, pdi, jj % 2, :],
                            pets[jj][:, sl],
                            start=False, stop=True,
                            tile_position=(0, 32 * jj))
                    # out = -(psum2 + DREAL/2); alternate engines so the
                    # epilogues drain in parallel
                    ot = opool.tile([C, 512], F32, name=f"ot_{jj}",
                                    tag="ot")
                    if jj % 2 == 0:
                        nc.vector.tensor_scalar(ot[:], psum2o[jj],
                                                float(DREAL) / 2.0, -1.0,
                                                op0=OP.add, op1=OP.mult)
                    else:
                        nc.scalar.activation(ot[:], psum2o[jj], AF.Copy,
                                             bias=-float(DREAL) / 2.0,
                                             scale=-1.0)
                    bb = GOFF[g] + jj
                    nc.gpsimd.dma_start(
                        out[:, bb * 512:(bb + 1) * 512], ot[:])
    nc.compile()
    _NC_CACHE["nc"] = nc
    return nc


def _prep_in_maps(samples, bhv_matrix, centroids):
    samples = np.ascontiguousarray(samples, dtype=np.float32)
    bhv_matrix = np.ascontiguousarray(bhv_matrix, dtype=np.float32)
    centroids = np.ascontiguousarray(centroids, dtype=np.float32)

    # x8 [128, NQ*2*B]: subgroup blobs back to back; within blob g,
    # [r, (q*2+t)*(nblk*512) + c] = fp8(64*(x-0.5))[f=q*256+t*128+r,
    # b = GOFF[g]*512 + c], 0-padded rows
    xz = np.zeros((FP, B), dtype=np.float32)
    xz[:F, :] = (samples.T - 0.5) * XSCALE
    xq = xz.reshape(NQ, 2, 128, B)
    blobs = []
    for g, nb in enumerate(GROUPS):
        b0 = GOFF[g] * 512
        xv = xq[:, :, :, b0:b0 + nb * 512].transpose(2, 0, 1, 3)
        blobs.append(np.ascontiguousarray(xv).reshape(128, -1))
    x_all = np.concatenate(blobs, axis=1).astype(NP_FP8)

    in_maps = []
    for k in range(NCORES):
        lo, hi = k * DREAL, (k + 1) * DREAL
        wz = np.zeros((FP, DP), dtype=np.float32)
        wz[:F, :DREAL] = bhv_matrix[lo:hi, :].T * XSCALE
        # SwInterleave stationary layout: [r, q, di, m, t] with m reversed,
        # flattened so position 2*(127-m)+t holds (f=q*256+t*128+r, d=di*128+m)
        wv = wz.reshape(NQ, 2, 128, ND, 128).transpose(2, 3, 0, 4, 1)
        wv = wv[:, :, :, ::-1, :]
        w8 = np.ascontiguousarray(wv).reshape(128, ND * NQ * 256)
        w8 = w8.astype(NP_FP8)
        cz = np.zeros((DP, 2, HC), dtype=np.float32)
        cz[:DREAL, 0, :C] = 1.0 - 2.0 * centroids[:, lo:hi].T
        cz[:, 1, :] = 0.5 * cz[:, 0, :]
        cv = cz.reshape(ND, 128, 2, HC).transpose(1, 0, 2, 3)
        cm8 = np.ascontiguousarray(cv).reshape(128, ND * 2 * HC)
        cm8 = cm8.astype(NP_FP8)
        in_maps.append({"x8": x_all, "w8": w8, "cm8": cm8})
    return in_maps


def _run(samples, bhv_matrix, centroids, **spmd_kwargs):
    nc = _build_nc()
    in_maps = _prep_in_maps(samples, bhv_matrix, centroids)
    res = run_bass_kernel_spmd(nc, in_maps, core_ids=list(range(NCORES)),
                               **spmd_kwargs)
    acc = np.zeros((C, B), dtype=np.float32)
    for r in res.results:
        acc += r["out"]
    return np.ascontiguousarray(acc.T), res


def kernel(samples, bhv_matrix, centroids):
    out, _ = _run(samples, bhv_matrix, centroids)
    return out


# revision 27
# speedup vs baseline: 1.0634x; 1.0408x over previous
"""Trainium2 Bass kernel for nn_BaselineMNISTClassifier (vq_codebook).

reference:
    x = samples - 0.5                        # [B, F]
    hv = einsum('bf,df->bd', x, bhv)         # [B, D]
    e = (hv > 0)                             # binary
    ham[b, c] = sum_d |e - centroids[c, d]|  # [B, C]
    return -ham

Only the SIGN of hv survives the binarize, so the encode matmul runs in
fp8 e4m3 (measured 0.91% bit-flip rate vs the f32 reference -> final
rel err ~8e-3, well under the 2e-2 gate). fp8 enables the PE's
DoubleRow perf mode: 256 contraction rows per matmul at 2 rows/cycle,
i.e. 4x the f32r rate the previous version used (256 cycles for a
256x128x512 matmul vs 512 cycles for 128x128x512).

Identity for the Hamming stage: with e' = (hv > 0) - 0.5 in {+-1/2} and
cmod = 1 - 2c in {-1, 0, +1}:  |e - c| = e' * cmod + 1/2, so
    ham[b, c] = sum_d e'[b, d] * cmod[c, d] + D/2
a second tiny matmul over the same d-tiles, also fp8 DoubleRow (exact:
all values are +-0.5/+-1/0). The binarize alternates DVE (is_gt-sub ->
e' = +-0.5) and ACT (Sign -> +-1) per d-tile; the matching 1.0 / 0.5
cmod scale is baked into the host-prepared centroid weights per d-tile
so both conventions contribute identically.

Sharding: D axis (10000) split across 8 cores, 1250 (padded 1280) per
core; every core sees the full batch, partial hammings sum on host.
F = 784 is zero-padded to 1024 = 4 chunks x (2 ktiles x 128 rows) so
every encode matmul is a uniform full-width DoubleRow op.

All quantization/transposition happens on host: x8 = fp8(64*(x-0.5)),
w8 = fp8(64*w) (the 64x scaling keeps values away from fp8 subnormals;
sign(hv) is scale-invariant), cmod in fp8 exactly.

Perf structure (per core):
  - warmup matmuls release the PE HAM clock gate while inputs stream
  - 2 b-groups of 4 blocks x 512; per (d-tile, block) 4 DoubleRow
    matmuls (q-chunks) accumulate in one PSUM bank; 7-bank rotation
  - consecutive matmuls over the 4 blocks share stationary weights,
    hiding LDWEIGHTS
  - hamming matmuls for d-pair P issue one d-tile late so the PE never
    waits on the binarize; all 4 accumulators of a b-group live in ONE
    PSUM bank at partition offsets 0/32/64/96 (tile_position)
  - last d-tile binarizes in halves; final hamming + epilogue drain
    per-block, outputs DMA out as their accumulation closes
"""

import sys

sys.path.insert(0, "/opt/trn_rl_repo")

import ml_dtypes
import numpy as np

import concourse.bacc as bacc
import concourse.bass as bass
import concourse.mybir as mybir
import concourse.tile as tile
from concourse.bass_utils import run_bass_kernel_spmd

B = 4096
F = 784
FP = 1024                    # F zero-padded: 4 chunks x (2 ktiles x 128)
NQ = 4                       # k-chunks of 256 (DoubleRow contraction)
D = 10000
C = 10
NCORES = 8
DREAL = D // NCORES          # 1250 real dims per core
DP = 1280                    # padded to 10 d-tiles of 128
ND = DP // 128               # 10
NPAIR = ND // 2              # 5 hamming d-pairs
NB = B // 512                # 8 b-blocks of 512
HC = 16                      # hamming stationary cols padded 10 -> 16
NWARM = 4                    # PE warmup matmuls
GROUPS = (4, 4)              # b-subgroup sizes in blocks of 512
GOFF = (0, 4)                # first block of each subgroup

F32 = mybir.dt.float32
FP8 = mybir.dt.float8e4
OP = mybir.AluOpType
AF = mybir.ActivationFunctionType
PM = mybir.MatmulPerfMode

NP_FP8 = ml_dtypes.float8_e4m3
XSCALE = 64.0

# binarize engine per b-block: even blocks on DVE (is_gt-sub, e' = +-0.5,
# cmod +-1), odd blocks on ACT (Sign, e2 = +-1, cmod +-0.5). The host
# provides both cmod scalings per d-tile so each hamming matmul picks the
# variant matching its block's binarize convention.

_NC_CACHE = {}


def _build_nc():
    if "nc" in _NC_CACHE:
        return _NC_CACHE["nc"]
    nc = bacc.Bacc("TRN2", debug=False, target_bir_lowering=False)
    # x8: per-subgroup blobs, each [128, NQ*2*nblk*512] partition-major
    # (contiguous -> one DMA descriptor per partition per blob)
    x8 = nc.dram_tensor("x8", [128, NQ * 2 * B], FP8,
                        kind="ExternalInput")
    # stationary operands come pre-interleaved for DoubleRowSwInterleave:
    # per partition, position 2*(M-1-m)+t holds the ktile-t weight of
    # output column m (the HW dual-fp8 LDWEIGHTS layout, probe-verified)
    w8 = nc.dram_tensor("w8", [128, ND * NQ * 256], FP8,
                        kind="ExternalInput")
    cm8 = nc.dram_tensor("cm8", [128, ND * 2 * HC], FP8,
                         kind="ExternalInput")
    out = nc.dram_tensor("out", [C, B], F32, kind="ExternalOutput")

    with tile.TileContext(nc) as tc:
        with (
            tc.tile_pool(name="dum", bufs=2) as dumpool,
            tc.tile_pool(name="xp", bufs=1) as xpool,
            tc.tile_pool(name="wp", bufs=1) as wpool,
            tc.tile_pool(name="cp", bufs=1) as cpool,
            tc.tile_pool(name="ep", bufs=12) as epool,
            tc.tile_pool(name="op", bufs=4) as opool,
            tc.tile_pool(name="pse", bufs=7, space="PSUM") as psepool,
            tc.tile_pool(name="ps2", bufs=1, space="PSUM") as ps2pool,
        ):
            # --- input loads: the first b-subgroup's x chunk (1MB) and
            # the first w chunk are the critical path to the first encode
            # matmul; everything else is deferred behind compute progress
            # (write/read dependencies keep the scheduler from hoisting
            # the triggers into the critical stream)
            xgs = [xpool.tile([128, NQ, 2, nb * 512], FP8, name=f"xg{g}",
                              tag=f"xt{g}")
                   for g, nb in enumerate(GROUPS)]

            def load_xg(g, eng):
                off = GOFF[g] * 512 * NQ * 2
                sz = GROUPS[g] * 512 * NQ * 2
                eng.dma_start(
                    xgs[g][:], x8[:, off:off + sz].rearrange(
                        "p (q t c) -> p q t c", q=NQ, t=2))

            # group 0 loads per q-chunk: the encode consumes q in order,
            # so the first d-tile starts as soon as chunk q0 lands and
            # runs DMA-paced through the remaining chunks
            qsz = GROUPS[0] * 512 * 2
            for q in range(NQ):
                nc.sync.dma_start(
                    xgs[0][:, q, :, :],
                    x8[:, q * qsz:(q + 1) * qsz].rearrange(
                        "p (t c) -> p t c", t=2))
            wt = wpool.tile([128, ND, NQ, 256], FP8)

            def load_w(lo, hi, eng):
                eng.dma_start(
                    wt[:, lo:hi, :, :],
                    w8[:, lo * NQ * 256:hi * NQ * 256].rearrange(
                        "p (a q m) -> p a q m", a=hi - lo, q=NQ))

            load_w(0, 4, nc.gpsimd)
            ct = cpool.tile([128, ND, 2, HC], FP8)
            nc.gpsimd.dma_start(
                ct[:], cm8.ap().rearrange("p (a v m) -> p a v m",
                                          a=ND, v=2))
            # w d4-9 waits on the xg0 load (1-elem copy creates the dep),
            # so it streams only after the critical chunk has landed
            nc.gpsimd.tensor_scalar_mul(wt[0:1, 4, 0, 0:1],
                                        xgs[0][0:1, 0, 0, 0:1], 0.0)
            load_w(4, ND, nc.gpsimd)

            # --- PE warmup: ramp the PE clock while inputs stream; the
            # dummies memset on DVE so the GpSimd queue stays free for
            # DMA triggers
            wdum = dumpool.tile([128, 256], FP8)
            nc.vector.memset(wdum[:], 1.0)
            xdum = dumpool.tile([128, 2, 512], FP8)
            nc.vector.memset(xdum[:], 1.0)
            psdum = psepool.tile([128, 512], F32, name="psdum", tag="pse")
            for i in range(NWARM):
                nc.tensor.matmul(psdum[:], wdum[:], xdum[:],
                                 start=(i == 0), stop=(i == NWARM - 1),
                                 perf_mode=PM.DoubleRowSwInterleave)

            # --- main compute: b-subgroups of GROUPS blocks of 512.
            for g, nblk in enumerate(GROUPS):
                ps2 = ps2pool.tile([128, 512], F32, name=f"ps2_{g % 2}",
                                   tag="ps2")
                psum2 = [ps2[32 * jj:32 * jj + HC, :] for jj in range(nblk)]
                psum2o = [ps2[32 * jj:32 * jj + C, :] for jj in range(nblk)]
                pending = []
                for di in range(ND):
                    pses = [
                        psepool.tile([128, 512], F32,
                                     name=f"pse_{di % 2}_{jj}", tag="pse")
                        for jj in range(nblk)
                    ]
                    for q in range(NQ):
                        wq = wt[:, di, q, :]
                        for jj in range(nblk):
                            nc.tensor.matmul(
                                pses[jj][:], wq,
                                xgs[g][:, q, :, jj * 512:(jj + 1) * 512],
                                start=(q == 0), stop=(q == NQ - 1),
                                perf_mode=PM.DoubleRowSwInterleave)
                    # hamming for the previous d-tile: issued here so the
                    # PE reaches it well after its binarize completes
                    for pdi, pets in pending:
                        for jj in range(nblk):
                            nc.tensor.matmul(
                                psum2[jj], ct[:, pdi, jj % 2, :],
                                pets[jj][:],
                                start=(pdi == 0), stop=False,
                                tile_position=(0, 32 * jj))
                    pending = []
                    # binarize this d-tile: even blocks on DVE (e'=+-0.5),
                    # odd blocks on ACT Sign (+-1); the last d-tile goes in
                    # halves so the final hamming overlaps
                    ets = [epool.tile([128, 512], FP8,
                                      name=f"et_{di % 2}_{jj}", tag="et")
                           for jj in range(nblk)]
                    for jj in range(nblk):
                        sls = ([slice(0, 256), slice(256, 512)]
                               if di == ND - 1 else [slice(0, 512)])
                        for sl in sls:
                            if jj % 2 == 0:
                                nc.vector.tensor_scalar(
                                    ets[jj][:, sl], pses[jj][:, sl],
                                    0.0, 0.5,
                                    op0=OP.is_gt, op1=OP.subtract)
                            else:
                                nc.scalar.activation(
                                    ets[jj][:, sl], pses[jj][:, sl],
                                    AF.Sign)
                    if g < len(GROUPS) - 1 and di == 1:
                        # release the next subgroup's x chunk only now: a
                        # 1-elem write into it (reading this d-tile's
                        # psum) gives its DMA a write dependency, keeping
                        # the scheduler from hoisting the trigger into the
                        # critical early stream
                        nc.vector.tensor_scalar_mul(
                            xgs[g + 1][0:1, 0, 0, 0:1],
                            pses[0][0:1, 0:1], 0.0)
                        load_xg(g + 1, nc.scalar)
                    pending.append((di, ets))
                # final d-tile: hamming in halves, epilogue + output DMA
                # per block as each accumulation closes
                pdi, pets = pending[0]
                for jj in range(nblk):
                    for h in range(2):
                        sl = slice(h * 256, (h + 1) * 256)
                        nc.tensor.matmul(
                            psum2[jj][:, sl], ct[:, pdi, jj % 2, :],
                            pets[jj][:, sl],
                            start=False, stop=True,
                            tile_position=(0, 32 * jj))
                    # out = -(psum2 + DREAL/2); alternate engines so the
                    # epilogues drain in parallel
                    ot = opool.tile([C, 512], F32, name=f"ot_{jj}",
                                    tag="ot")
                    if jj % 2 == 0:
                        nc.vector.tensor_scalar(ot[:], psum2o[jj],
                                                float(DREAL) / 2.0, -1.0,
                                                op0=OP.add, op1=OP.mult)
                    else:
                        nc.scalar.activation(ot[:], psum2o[jj], AF.Copy,
                                             bias=-float(DREAL) / 2.0,
                                             scale=-1.0)
                    bb = GOFF[g] + jj
                    (nc.gpsimd if jj % 2 == 0 else nc.sync).dma_start(
                        out[:, bb * 512:(bb + 1) * 512], ot[:])
    nc.compile()
    _NC_CACHE["nc"] = nc
    return nc


def _prep_in_maps(samples, bhv_matrix, centroids):
    samples = np.ascontiguousarray(samples, dtype=np.float32)
    bhv_matrix = np.ascontiguousarray(bhv_matrix, dtype=np.float32)
    centroids = np.ascontiguousarray(centroids, dtype=np.float32)

    # x8 [128, NQ*2*B]: subgroup blobs back to back; within blob g,
    # [r, (q*2+t)*(nblk*512) + c] = fp8(64*(x-0.5))[f=q*256+t*128+r,
    # b = GOFF[g]*512 + c], 0-padded rows
    xz = np.zeros((FP, B), dtype=np.float32)
    xz[:F, :] = (samples.T - 0.5) * XSCALE
    xq = xz.reshape(NQ, 2, 128, B)
    blobs = []
    for g, nb in enumerate(GROUPS):
        b0 = GOFF[g] * 512
        xv = xq[:, :, :, b0:b0 + nb * 512].transpose(2, 0, 1, 3)
        blobs.append(np.ascontiguousarray(xv).reshape(128, -1))
    x_all = np.concatenate(blobs, axis=1).astype(NP_FP8)

    in_maps = []
    for k in range(NCORES):
        lo, hi = k * DREAL, (k + 1) * DREAL
        wz = np.zeros((FP, DP), dtype=np.float32)
        wz[:F, :DREAL] = bhv_matrix[lo:hi, :].T * XSCALE
        # SwInterleave stationary layout: [r, q, di, m, t] with m reversed,
        # flattened so position 2*(127-m)+t holds (f=q*256+t*128+r, d=di*128+m)
        wv = wz.reshape(NQ, 2, 128, ND, 128).transpose(2, 3, 0, 4, 1)
        wv = wv[:, :, :, ::-1, :]
        w8 = np.ascontiguousarray(wv).reshape(128, ND * NQ * 256)
        w8 = w8.astype(NP_FP8)
        cz = np.zeros((DP, 2, HC), dtype=np.float32)
        cz[:DREAL, 0, :C] = 1.0 - 2.0 * centroids[:, lo:hi].T
        cz[:, 1, :] = 0.5 * cz[:, 0, :]
        cv = cz.reshape(ND, 128, 2, HC).transpose(1, 0, 2, 3)
        cm8 = np.ascontiguousarray(cv).reshape(128, ND * 2 * HC)
        cm8 = cm8.astype(NP_FP8)
        in_maps.append({"x8": x_all, "w8": w8, "cm8": cm8})
    return in_maps


def _run(samples, bhv_matrix, centroids, **spmd_kwargs):
    nc = _build_nc()
    in_maps = _prep_in_maps(samples, bhv_matrix, centroids)
    res = run_bass_kernel_spmd(nc, in_maps, core_ids=list(range(NCORES)),
                               **spmd_kwargs)
    acc = np.zeros((C, B), dtype=np.float32)
    for r in res.results:
        acc += r["out"]
    return np.ascontiguousarray(acc.T), res


def kernel(samples, bhv_matrix, centroids):
    out, _ = _run(samples, bhv_matrix, centroids)
    return out


# revision 29
# speedup vs baseline: 1.0641x; 1.0007x over previous
"""Trainium2 Bass kernel for nn_BaselineMNISTClassifier (vq_codebook).

reference:
    x = samples - 0.5                        # [B, F]
    hv = einsum('bf,df->bd', x, bhv)         # [B, D]
    e = (hv > 0)                             # binary
    ham[b, c] = sum_d |e - centroids[c, d]|  # [B, C]
    return -ham

Only the SIGN of hv survives the binarize, so the encode matmul runs in
fp8 e4m3 (measured 0.91% bit-flip rate vs the f32 reference -> final
rel err 7.9e-3, well under the 2e-2 gate). fp8 enables the PE's
DoubleRowSwInterleave perf mode: 256 contraction rows per 512-cycle
matmul instruction -- 2x the MAC rate of the f32r version this
replaces. Encode is at the dual-fp8 PE roofline: 320 matmuls x 216 ns.

The stationary operands are host-pre-interleaved in the dual-fp8
LDWEIGHTS layout (A/B k-tile pairs adjacent per output column, columns
reversed: position 2*(M-1-m)+t holds the ktile-t weight of output
column m) -- verified against hardware with an identity-probe kernel.

Hamming stage identity: with e' = (hv > 0) - 0.5 in {+-1/2} and
cmod = 1 - 2c in {-1, 0, +1}:  |e - c| = e' * cmod + 1/2, so
    ham[b, c] = sum_d e'[b, d] * cmod[c, d] + D/2
a second tiny fp8 matmul over the same d-tiles (exact: all values
+-0.5/+-1/0). The binarize alternates engines per block: even blocks
DVE is_gt-subtract (e' = +-0.5), odd blocks ACT Sign (+-1); the host
provides both cmod scalings (x1 and x0.5) per d-tile so each hamming
matmul picks the variant matching its block's convention. (A paired
DoubleRow hamming hits a walrus codegen bug -- dual-fp8 LDWEIGHTS with
tile_position emits inconsistent num_active_cols -- so the hamming
stays in plain fp8, ~10 us of the ~85 us PE budget.)

Sharding: D axis (10000) split across 8 cores, 1250 (padded 1280) per
core; every core sees the full batch, partial hammings sum on host.
F = 784 is zero-padded to 1024 = 4 chunks x (2 ktiles x 128 rows); the
pad costs no PE time (instruction cost is per moving column).

Host does all quantization/transposition: x8 = fp8(64*(x-0.5)),
w8 = fp8(64*w) (64x keeps values out of fp8 subnormals; sign(hv) is
scale-invariant), cmod exact in fp8.

Perf structure (per core, ~105 us measured):
  - all input layouts are partition-major blobs so every DMA descriptor
    is one contiguous multi-KB run per partition (descriptor-count,
    not bandwidth, limits these transfers otherwise)
  - group 0's x streams per q-chunk: the first d-tile starts as soon as
    chunk q0 lands and runs DMA-paced behind the remaining chunks,
    masking the PE clock-gate ramp; 4 warmup matmuls ahead of it
  - non-critical loads (later x/w chunks) are released by 1-element
    copies that read early compute results: the write dependency stops
    the tile scheduler from hoisting their triggers into the critical
    first stream
  - 2 b-groups of 4 blocks x 512; per (d-tile, block) 4 DoubleRow
    matmuls accumulate in one PSUM bank; 7-bank rotation; 4 consecutive
    matmuls share stationary weights, hiding the ~270 ns LDWEIGHTS
    (2- and 3-block groups measurably stall the PE on weight loads)
  - hamming matmuls issue one d-tile late so the PE never waits on the
    binarize; all 4 accumulators of a group live in ONE PSUM bank at
    partition offsets 0/32/64/96 (tile_position)
  - last d-tile binarizes in quarters across both engines; final
    hamming + epilogue drain per-block, each block's output DMAs out
    (gpsimd/sync alternating) as its accumulation closes
"""

import sys

sys.path.insert(0, "/opt/trn_rl_repo")

import ml_dtypes
import numpy as np

import concourse.bacc as bacc
import concourse.bass as bass
import concourse.mybir as mybir
import concourse.tile as tile
from concourse.bass_utils import run_bass_kernel_spmd

B = 4096
F = 784
FP = 1024                    # F zero-padded: 4 chunks x (2 ktiles x 128)
NQ = 4                       # k-chunks of 256 (DoubleRow contraction)
D = 10000
C = 10
NCORES = 8
DREAL = D // NCORES          # 1250 real dims per core
DP = 1280                    # padded to 10 d-tiles of 128
ND = DP // 128               # 10
NPAIR = ND // 2              # 5 hamming d-pairs
NB = B // 512                # 8 b-blocks of 512
HC = 16                      # hamming stationary cols padded 10 -> 16
NWARM = 4                    # PE warmup matmuls
GROUPS = (4, 4)              # b-subgroup sizes in blocks of 512
GOFF = (0, 4)                # first block of each subgroup

F32 = mybir.dt.float32
FP8 = mybir.dt.float8e4
OP = mybir.AluOpType
AF = mybir.ActivationFunctionType
PM = mybir.MatmulPerfMode

NP_FP8 = ml_dtypes.float8_e4m3
XSCALE = 64.0

# binarize engine per b-block: even blocks on DVE (is_gt-sub, e' = +-0.5,
# cmod +-1), odd blocks on ACT (Sign, e2 = +-1, cmod +-0.5). The host
# provides both cmod scalings per d-tile so each hamming matmul picks the
# variant matching its block's binarize convention.

_NC_CACHE = {}


def _build_nc():
    if "nc" in _NC_CACHE:
        return _NC_CACHE["nc"]
    nc = bacc.Bacc("TRN2", debug=False, target_bir_lowering=False)
    # x8: per-subgroup blobs, each [128, NQ*2*nblk*512] partition-major
    # (contiguous -> one DMA descriptor per partition per blob)
    x8 = nc.dram_tensor("x8", [128, NQ * 2 * B], FP8,
                        kind="ExternalInput")
    # stationary operands come pre-interleaved for DoubleRowSwInterleave:
    # per partition, position 2*(M-1-m)+t holds the ktile-t weight of
    # output column m (the HW dual-fp8 LDWEIGHTS layout, probe-verified)
    w8 = nc.dram_tensor("w8", [128, ND * NQ * 256], FP8,
                        kind="ExternalInput")
    cm8 = nc.dram_tensor("cm8", [128, ND * 2 * HC], FP8,
                         kind="ExternalInput")
    out = nc.dram_tensor("out", [C, B], F32, kind="ExternalOutput")

    with tile.TileContext(nc) as tc:
        with (
            tc.tile_pool(name="dum", bufs=2) as dumpool,
            tc.tile_pool(name="xp", bufs=1) as xpool,
            tc.tile_pool(name="wp", bufs=1) as wpool,
            tc.tile_pool(name="cp", bufs=1) as cpool,
            tc.tile_pool(name="ep", bufs=12) as epool,
            tc.tile_pool(name="op", bufs=4) as opool,
            tc.tile_pool(name="pse", bufs=7, space="PSUM") as psepool,
            tc.tile_pool(name="ps2", bufs=1, space="PSUM") as ps2pool,
        ):
            # --- input loads: the first b-subgroup's x chunk (1MB) and
            # the first w chunk are the critical path to the first encode
            # matmul; everything else is deferred behind compute progress
            # (write/read dependencies keep the scheduler from hoisting
            # the triggers into the critical stream)
            xgs = [xpool.tile([128, NQ, 2, nb * 512], FP8, name=f"xg{g}",
                              tag=f"xt{g}")
                   for g, nb in enumerate(GROUPS)]

            def load_xg(g, eng):
                off = GOFF[g] * 512 * NQ * 2
                sz = GROUPS[g] * 512 * NQ * 2
                eng.dma_start(
                    xgs[g][:], x8[:, off:off + sz].rearrange(
                        "p (q t c) -> p q t c", q=NQ, t=2))

            # group 0 loads per q-chunk: the encode consumes q in order,
            # so the first d-tile starts as soon as chunk q0 lands and
            # runs DMA-paced through the remaining chunks
            qsz = GROUPS[0] * 512 * 2
            for q in range(NQ):
                nc.sync.dma_start(
                    xgs[0][:, q, :, :],
                    x8[:, q * qsz:(q + 1) * qsz].rearrange(
                        "p (t c) -> p t c", t=2))
            wt = wpool.tile([128, ND, NQ, 256], FP8)

            def load_w(lo, hi, eng):
                eng.dma_start(
                    wt[:, lo:hi, :, :],
                    w8[:, lo * NQ * 256:hi * NQ * 256].rearrange(
                        "p (a q m) -> p a q m", a=hi - lo, q=NQ))

            load_w(0, 2, nc.gpsimd)
            load_w(2, 4, nc.gpsimd)
            ct = cpool.tile([128, ND, 2, HC], FP8)
            nc.gpsimd.dma_start(
                ct[:], cm8.ap().rearrange("p (a v m) -> p a v m",
                                          a=ND, v=2))
            # w d4-9 waits on the xg0 load (1-elem copy creates the dep),
            # so it streams only after the critical chunk has landed
            nc.gpsimd.tensor_scalar_mul(wt[0:1, 4, 0, 0:1],
                                        xgs[0][0:1, 0, 0, 0:1], 0.0)
            load_w(4, ND, nc.gpsimd)

            # --- PE warmup: ramp the PE clock while inputs stream; the
            # dummies memset on DVE so the GpSimd queue stays free for
            # DMA triggers
            wdum = dumpool.tile([128, 256], FP8)
            nc.vector.memset(wdum[:], 1.0)
            xdum = dumpool.tile([128, 2, 512], FP8)
            nc.vector.memset(xdum[:], 1.0)
            psdum = psepool.tile([128, 512], F32, name="psdum", tag="pse")
            for i in range(NWARM):
                nc.tensor.matmul(psdum[:], wdum[:], xdum[:],
                                 start=(i == 0), stop=(i == NWARM - 1),
                                 perf_mode=PM.DoubleRowSwInterleave)

            # --- main compute: b-subgroups of GROUPS blocks of 512.
            for g, nblk in enumerate(GROUPS):
                ps2 = ps2pool.tile([128, 512], F32, name=f"ps2_{g % 2}",
                                   tag="ps2")
                psum2 = [ps2[32 * jj:32 * jj + HC, :] for jj in range(nblk)]
                psum2o = [ps2[32 * jj:32 * jj + C, :] for jj in range(nblk)]
                pending = []
                for di in range(ND):
                    pses = [
                        psepool.tile([128, 512], F32,
                                     name=f"pse_{di % 2}_{jj}", tag="pse")
                        for jj in range(nblk)
                    ]
                    for q in range(NQ):
                        wq = wt[:, di, q, :]
                        for jj in range(nblk):
                            nc.tensor.matmul(
                                pses[jj][:], wq,
                                xgs[g][:, q, :, jj * 512:(jj + 1) * 512],
                                start=(q == 0), stop=(q == NQ - 1),
                                perf_mode=PM.DoubleRowSwInterleave)
                    # hamming for the previous d-tile: issued here so the
                    # PE reaches it well after its binarize completes
                    for pdi, pets in pending:
                        for jj in range(nblk):
                            nc.tensor.matmul(
                                psum2[jj], ct[:, pdi, jj % 2, :],
                                pets[jj][:],
                                start=(pdi == 0), stop=False,
                                tile_position=(0, 32 * jj))
                    pending = []
                    # binarize this d-tile: even blocks on DVE (e'=+-0.5),
                    # odd blocks on ACT Sign (+-1); the last d-tile goes in
                    # halves so the final hamming overlaps
                    ets = [epool.tile([128, 512], FP8,
                                      name=f"et_{di % 2}_{jj}", tag="et")
                           for jj in range(nblk)]
                    for jj in range(nblk):
                        sls = ([slice(k * 128, (k + 1) * 128)
                                for k in range(4)]
                               if di == ND - 1 else [slice(0, 512)])
                        for sl in sls:
                            if jj % 2 == 0:
                                nc.vector.tensor_scalar(
                                    ets[jj][:, sl], pses[jj][:, sl],
                                    0.0, 0.5,
                                    op0=OP.is_gt, op1=OP.subtract)
                            else:
                                nc.scalar.activation(
                                    ets[jj][:, sl], pses[jj][:, sl],
                                    AF.Sign)
                    if g < len(GROUPS) - 1 and di == 1:
                        # release the next subgroup's x chunk only now: a
                        # 1-elem write into it (reading this d-tile's
                        # psum) gives its DMA a write dependency, keeping
                        # the scheduler from hoisting the trigger into the
                        # critical early stream
                        nc.vector.tensor_scalar_mul(
                            xgs[g + 1][0:1, 0, 0, 0:1],
                            pses[0][0:1, 0:1], 0.0)
                        load_xg(g + 1, nc.scalar)
                    pending.append((di, ets))
                # final d-tile: hamming in halves, epilogue + output DMA
                # per block as each accumulation closes
                pdi, pets = pending[0]
                for jj in range(nblk):
                    for h in range(4):
                        sl = slice(h * 128, (h + 1) * 128)
                        nc.tensor.matmul(
                            psum2[jj][:, sl], ct[:, pdi, jj % 2, :],
                            pets[jj][:, sl],
                            start=False, stop=True,
                            tile_position=(0, 32 * jj))
                    # out = -(psum2 + DREAL/2); alternate engines so the
                    # epilogues drain in parallel
                    ot = opool.tile([C, 512], F32, name=f"ot_{jj}",
                                    tag="ot")
                    if jj % 2 == 0:
                        nc.vector.tensor_scalar(ot[:], psum2o[jj],
                                                float(DREAL) / 2.0, -1.0,
                                                op0=OP.add, op1=OP.mult)
                    else:
                        nc.scalar.activation(ot[:], psum2o[jj], AF.Copy,
                                             bias=-float(DREAL) / 2.0,
                                             scale=-1.0)
                    bb = GOFF[g] + jj
                    (nc.gpsimd if jj % 2 == 0 else nc.sync).dma_start(
                        out[:, bb * 512:(bb + 1) * 512], ot[:])
    nc.compile()
    _NC_CACHE["nc"] = nc
    return nc


def _prep_in_maps(samples, bhv_matrix, centroids):
    samples = np.ascontiguousarray(samples, dtype=np.float32)
    bhv_matrix = np.ascontiguousarray(bhv_matrix, dtype=np.float32)
    centroids = np.ascontiguousarray(centroids, dtype=np.float32)

    # x8 [128, NQ*2*B]: subgroup blobs back to back; within blob g,
    # [r, (q*2+t)*(nblk*512) + c] = fp8(64*(x-0.5))[f=q*256+t*128+r,
    # b = GOFF[g]*512 + c], 0-padded rows
    xz = np.zeros((FP, B), dtype=np.float32)
    xz[:F, :] = (samples.T - 0.5) * XSCALE
    xq = xz.reshape(NQ, 2, 128, B)
    blobs = []
    for g, nb in enumerate(GROUPS):
        b0 = GOFF[g] * 512
        xv = xq[:, :, :, b0:b0 + nb * 512].transpose(2, 0, 1, 3)
        blobs.append(np.ascontiguousarray(xv).reshape(128, -1))
    x_all = np.concatenate(blobs, axis=1).astype(NP_FP8)

    in_maps = []
    for k in range(NCORES):
        lo, hi = k * DREAL, (k + 1) * DREAL
        wz = np.zeros((FP, DP), dtype=np.float32)
        wz[:F, :DREAL] = bhv_matrix[lo:hi, :].T * XSCALE
        # SwInterleave stationary layout: [r, q, di, m, t] with m reversed,
        # flattened so position 2*(127-m)+t holds (f=q*256+t*128+r, d=di*128+m)
        wv = wz.reshape(NQ, 2, 128, ND, 128).transpose(2, 3, 0, 4, 1)
        wv = wv[:, :, :, ::-1, :]
        w8 = np.ascontiguousarray(wv).reshape(128, ND * NQ * 256)
        w8 = w8.astype(NP_FP8)
        cz = np.zeros((DP, 2, HC), dtype=np.float32)
        cz[:DREAL, 0, :C] = 1.0 - 2.0 * centroids[:, lo:hi].T
        cz[:, 1, :] = 0.5 * cz[:, 0, :]
        cv = cz.reshape(ND, 128, 2, HC).transpose(1, 0, 2, 3)
        cm8 = np.ascontiguousarray(cv).reshape(128, ND * 2 * HC)
        cm8 = cm8.astype(NP_FP8)
        in_maps.append({"x8": x_all, "w8": w8, "cm8": cm8})
    return in_maps


def _run(samples, bhv_matrix, centroids, **spmd_kwargs):
    nc = _build_nc()
    in_maps = _prep_in_maps(samples, bhv_matrix, centroids)
    res = run_bass_kernel_spmd(nc, in_maps, core_ids=list(range(NCORES)),
                               **spmd_kwargs)
    acc = np.zeros((C, B), dtype=np.float32)
    for r in res.results:
        acc += r["out"]
    return np.ascontiguousarray(acc.T), res


def kernel(samples, bhv_matrix, centroids):
    out, _ = _run(samples, bhv_matrix, centroids)
    return out
